# revision 11
# baseline (speedup 1.0000x reference)
"""Trainium2 Bass kernel for AdaptConv-style GNN message passing.

Reference computation (per batch element b):
    h   = x @ W.T + b                       # [N, OUT]
    hn  = h / max(||h||_row, 1e-12)         # row-wise L2 normalize
    cos = hn @ hn.T                         # [N, N]
    out = relu((edge_weight * cos) @ h)     # [N, OUT]

Algebraic restructure used on-chip (r_p = 1/max(||h_p||, eps)):
    out[p,:] = r_p * sum_q E[p,q] * S[p,q] * hn[q,:]
with S = h h^T the UNNORMALIZED gram.  The gram operands need no
pre-normalization (no hnT materialization, half the PE transposes); the
row scale r_p is applied in a cheap bf16 epilogue against a
row-replicated copy of r built via DMA-xbar transpose + DRAM broadcast.

Sharding: pure data-parallel over batch B=8 across 8 NeuronCores.  Host
layout preprocessing: et = edge_weight[b].T (bf16), xt = x[b].T (bf16),
wt = W.T (bf16), bias = b[:,None] (fp32); output returns as
outT = [OUT, N] bf16 and the host transposes/casts to fp32.

Per-core dataflow (fp32 PSUM accumulation everywhere):
    hT[o,n]    = wt.T @ xt + bias          (PE, 2x1024 chunks + ScalarE bias)
    h tiles    = PE-transpose(hT) -> GpSimd copy to SBUF bf16 (16x)
    norms      = ScalarE Square-accum -> Sqrt -> max/recip (chunked)
    hn8 pairs  = h * r_inv -> fp8 [128, 2*OUT] interleaved (agg weights)
    per band q (16): 4 raw-gram matmuls [128,512] (1 hT-block LDW each 4)
        gates gt8 = et * S -> fp8, split: DVE-direct / GpSimd-direct /
        2x(ScalarE psum->bf16 copy -> DVE bf16 mul)
    per band pair: 4 DoubleRow fp8 matmuls accumulate outT [OUT,2048]
    epilogue: ScalarE relu -> DVE *rrep (bf16) -> DMA out per 512 chunk

The PE is fed dummy warm-up transposes first so its DVFS p-state ramps
to 2.4 GHz before the gram matmuls start, and is kept busy end-to-end.
"""

import ml_dtypes
import numpy as np

import concourse.bass as bass
import concourse.mybir as mybir
import concourse.tile as tile
from concourse import bacc
from concourse.bass_utils import run_bass_kernel_spmd
from concourse.masks import make_identity

B, N, IN, OUT = 8, 2048, 128, 128
NQ = N // 128          # 16 row bands
NP = NQ // 2           # 8 band pairs (DoubleRow)
FP32 = mybir.dt.float32
BF16 = mybir.dt.bfloat16
FP8 = mybir.dt.float8e4
AF = mybir.ActivationFunctionType
EPS = 1e-12
WARMUP = 14            # dummy PE transposes to ramp the p-state
USE_FP8 = True         # fp8 gt/hn + DoubleRow agg (2x PE on the agg matmul)

CORE_IDS = list(range(8))


def build_nc():
    from contextlib import ExitStack

    nc = bacc.Bacc("TRN2", target_bir_lowering=False, debug=False, num_devices=8)

    et = nc.dram_tensor("et", [N, N], BF16, kind="ExternalInput").ap()
    xt = nc.dram_tensor("xt", [IN, N], BF16, kind="ExternalInput").ap()
    wt = nc.dram_tensor("wt", [IN, OUT], BF16, kind="ExternalInput").ap()
    bias = nc.dram_tensor("bias", [OUT, 1], FP32, kind="ExternalInput").ap()
    out = nc.dram_tensor("out", [OUT, N], BF16, kind="ExternalOutput").ap()
    rr_d = nc.dram_tensor("rr_d", [1, N], BF16, kind="Internal").ap()

    with tile.TileContext(nc) as tc, ExitStack() as ctx:
        singles = ctx.enter_context(tc.tile_pool(name="singles", bufs=1))
        # et stream pool directly after singles so its range never aliases
        # transient scratch (no WAR deps on the stream).
        etp = ctx.enter_context(tc.tile_pool(name="etp", bufs=16))
        gt8p = ctx.enter_context(tc.tile_pool(name="gt8p", bufs=3))
        csbp = ctx.enter_context(tc.tile_pool(name="csbp", bufs=6))
        sqp = ctx.enter_context(tc.tile_pool(name="sqp", bufs=2))

        ident = singles.tile([128, 128], BF16, tag="ident")
        make_identity(nc, ident[:])

        xt_sb = singles.tile([IN, N], BF16, tag="xt_sb")
        wt_sb = singles.tile([IN, OUT], BF16, tag="wt_sb")
        bias_sb = singles.tile([OUT, 1], FP32, tag="bias_sb")
        hT = singles.tile([128, N], BF16, tag="hT")
        hrm = [
            singles.tile([128, 128], BF16, tag=f"hrm{i}", name=f"hrm{i}")
            for i in range(NQ)
        ]
        gdt = FP8 if USE_FP8 else BF16
        hn8 = [
            singles.tile([128, 2 * OUT], gdt, tag=f"hn8_{i}", name=f"hn8_{i}")
            for i in range(NP)
        ]
        s_acc = singles.tile([128, NQ], FP32, tag="s_acc")
        s_nrm = singles.tile([128, NQ], FP32, tag="s_nrm")
        s_max = singles.tile([128, NQ], FP32, tag="s_max")
        r_inv = singles.tile([128, NQ], FP32, tag="r_inv")
        r_pad = singles.tile([128, 128], BF16, tag="r_pad")
        tpr = singles.tile([128, 128], BF16, tag="tpr")
        rrep = singles.tile([128, N], BF16, tag="rrep")
        out_r = singles.tile([OUT, N], BF16, tag="out_r")
        out_sb = singles.tile([OUT, N], BF16, tag="out_sb")

        # scalar Sqrt table warm + r_pad zero, off the critical path
        dumm = sqp.tile([1, 2], FP32, tag="dumm")
        nc.gpsimd.memset(dumm[:], 1.0)
        dumm2 = sqp.tile([1, 2], FP32, tag="dumm2")
        nc.scalar.activation(dumm2[:], dumm[:], AF.Sqrt)
        nc.gpsimd.memset(r_pad[:], 0.0)

        # params head the sync DMA queue, then the et band stream
        nc.sync.dma_start(xt_sb[:], xt)
        nc.sync.dma_start(wt_sb[:], wt)
        nc.sync.dma_start(bias_sb[:], bias)
        etbs = []
        for q in range(NQ):
            etb = etp.tile([128, N], BF16, tag="etb", name=f"etb{q}")
            nc.sync.dma_start(etb[:], et[q * 128 : (q + 1) * 128, :])
            etbs.append(etb)

        # ---------- prologue psum (scoped; closes before main pools) ----------
        with ExitStack() as pctx:
            warm = pctx.enter_context(tc.tile_pool(name="warm", bufs=1, space="PSUM"))
            hps_pool = pctx.enter_context(
                tc.tile_pool(name="hps", bufs=3, space="PSUM")
            )
            tpp = pctx.enter_context(tc.tile_pool(name="tpp", bufs=4, space="PSUM"))

            # PE p-state warm-up while waiting for xt: harmless transposes
            wtile = warm.tile([128, 128], BF16, tag="wtile")
            for _ in range(WARMUP):
                nc.tensor.transpose(wtile[:], ident[:], ident[:])

            # hT = wt.T @ xt + bias (four 512-wide chunks; first gram matmul
            # can start as soon as bias chunk 0 lands)
            hps = []
            for c in range(3):
                ps = hps_pool.tile([OUT, 512], FP32, tag="hps", name=f"hps{c}")
                nc.tensor.matmul(
                    ps[:], wt_sb[:], xt_sb[:, c * 512 : (c + 1) * 512],
                    start=True, stop=True,
                )
                hps.append(ps)
            # keep the PE p-state alive while the first bias chunk drains
            for _ in range(3):
                nc.tensor.transpose(wtile[:], ident[:], ident[:])
            ps3 = hps_pool.tile([OUT, 512], FP32, tag="hps", name="hps3")
            nc.tensor.matmul(
                ps3[:], wt_sb[:], xt_sb[:, 3 * 512 : 4 * 512], start=True, stop=True
            )
            hps.append(ps3)
            for c in range(4):
                nc.scalar.activation(
                    hT[:, c * 512 : (c + 1) * 512], hps[c][:], AF.Identity,
                    bias=bias_sb[:], scale=1.0,
                )

            # row-major h tiles: PE transpose -> DVE/ScalarE copy to SBUF bf16
            for i in range(NQ):
                tp = tpp.tile([128, 128], BF16, tag="tp", name=f"tp{i}")
                nc.tensor.transpose(tp[:], hT[:, i * 128 : (i + 1) * 128], ident[:])
                if i < 8:
                    nc.vector.tensor_copy(hrm[i][:], tp[:])
                else:
                    nc.scalar.copy(hrm[i][:], tp[:])

        # ---------- main psum pools ----------
        cosp = ctx.enter_context(tc.tile_pool(name="cosp", bufs=4, space="PSUM"))
        outp = ctx.enter_context(tc.tile_pool(name="outp", bufs=1, space="PSUM"))
        outTs = [
            outp.tile([OUT, 512], FP32, tag=f"outT{c}", name=f"outT{c}")
            for c in range(4)
        ]

        SAFE_OPS = True

        def emit_sq(i):
            sq = sqp.tile([128, OUT], BF16, tag="sq", name=f"sq{i}")
            if SAFE_OPS:
                nc.scalar.activation(
                    sq[:], hrm[i][:], AF.Square, accum_out=s_acc[:, i : i + 1]
                )
            else:
                nc.vector.tensor_tensor_reduce(
                    sq[:], hrm[i][:], hrm[i][:], 1.0, 0.0,
                    mybir.AluOpType.mult, mybir.AluOpType.add,
                    accum_out=s_acc[:, i : i + 1],
                )

        def emit_rfin(ck):
            nc.scalar.activation(s_nrm[:, ck], s_acc[:, ck], AF.Sqrt)
            nc.vector.tensor_scalar_max(s_max[:, ck], s_nrm[:, ck], EPS)
            nc.vector.reciprocal(r_inv[:, ck], s_max[:, ck])

        def emit_hn8(i):
            eng = nc.vector if SAFE_OPS else nc.gpsimd
            eng.tensor_scalar_mul(
                hn8[i // 2][:, (i % 2) * OUT : (i % 2 + 1) * OUT],
                hrm[i][:], r_inv[:, i : i + 1],
            )

        gt8s = [None] * NP

        def emit_band(q):
            """raw-gram matmuls + gates for band q."""
            if q % 2 == 0:
                gt8s[q // 2] = gt8p.tile([128, 2 * N], gdt, tag="gt8", name=f"gt8_{q//2}")
            gt = gt8s[q // 2]
            ko = (q % 2) * N
            cps = []
            for c in range(4):
                cp = cosp.tile([128, 512], FP32, tag="cps", name=f"cps{q}_{c}")
                nc.tensor.matmul(
                    cp[:], hT[:, q * 128 : (q + 1) * 128],
                    hT[:, c * 512 : (c + 1) * 512],
                    start=True, stop=True,
                )
                cps.append(cp)
            # chunk 0: DVE direct from psum
            nc.vector.tensor_mul(
                gt[:, ko : ko + 512], cps[0][:], etbs[q][:, 0:512]
            )
            # chunk 1: ScalarE psum->bf16 copy, then GpSimd bf16 mul
            csb1 = csbp.tile([128, 512], BF16, tag="csb", name=f"csb1_{q}")
            nc.scalar.copy(csb1[:], cps[1][:])
            nc.gpsimd.tensor_mul(
                gt[:, ko + 512 : ko + 1024], csb1[:], etbs[q][:, 512:1024]
            )
            # chunk 2: ScalarE psum->bf16 copy, then DVE bf16 mul
            csb2 = csbp.tile([128, 512], BF16, tag="csb", name=f"csb2_{q}")
            nc.scalar.copy(csb2[:], cps[2][:])
            nc.vector.tensor_mul(
                gt[:, ko + 1024 : ko + 1536], csb2[:], etbs[q][:, 1024:1536]
            )
            # chunk 3: rotate V-direct / ScalarE+DVE / ScalarE+GpSimd
            if q % 3 == 0:
                nc.vector.tensor_mul(
                    gt[:, ko + 1536 : ko + 2048], cps[3][:], etbs[q][:, 1536:2048]
                )
            else:
                csb3 = csbp.tile([128, 512], BF16, tag="csb", name=f"csb3_{q}")
                nc.scalar.copy(csb3[:], cps[3][:])
                if q % 3 == 1:
                    nc.vector.tensor_mul(
                        gt[:, ko + 1536 : ko + 2048], csb3[:], etbs[q][:, 1536:2048]
                    )
                else:
                    nc.gpsimd.tensor_mul(
                        gt[:, ko + 1536 : ko + 2048], csb3[:], etbs[q][:, 1536:2048]
                    )

        def emit_agg(p):
            if USE_FP8:
                lhs = hn8[p][:].rearrange("q (k m) -> q k m", k=2)
                rhs = gt8s[p][:].rearrange("q (k n) -> q k n", k=2)
                for c in range(4):
                    nc.tensor.matmul(
                        outTs[c][:], lhs, rhs[:, :, c * 512 : (c + 1) * 512],
                        start=(p == 0), stop=(p == NP - 1),
                        perf_mode=mybir.MatmulPerfMode.DoubleRow,
                    )
            else:
                for ko in range(2):
                    for c in range(4):
                        nc.tensor.matmul(
                            outTs[c][:],
                            hn8[p][:, ko * OUT : (ko + 1) * OUT],
                            gt8s[p][:, ko * N + c * 512 : ko * N + (c + 1) * 512],
                            start=(p == 0 and ko == 0),
                            stop=(p == NP - 1 and ko == 1),
                        )

        emit_band(0)
        emit_sq(0)
        emit_sq(1)
        emit_rfin(slice(0, 2))
        emit_hn8(0)
        emit_hn8(1)
        emit_band(1)
        emit_sq(2)
        emit_sq(3)
        emit_band(2)
        emit_agg(0)
        emit_rfin(slice(2, 4))
        emit_hn8(2)
        emit_hn8(3)
        emit_band(3)
        emit_sq(4)
        emit_sq(5)
        emit_band(4)
        emit_agg(1)
        emit_sq(6)
        emit_sq(7)
        emit_band(5)
        emit_rfin(slice(4, 8))
        for i in range(4, 8):
            emit_hn8(i)
        emit_sq(8)
        emit_sq(9)
        emit_band(6)
        emit_agg(2)
        emit_sq(10)
        emit_sq(11)
        emit_band(7)
        emit_sq(12)
        emit_sq(13)
        emit_band(8)
        emit_agg(3)
        emit_sq(14)
        emit_sq(15)
        emit_band(9)
        emit_rfin(slice(8, 16))
        for i in range(8, 16):
            emit_hn8(i)
        # rrep: r_inv -> bf16 (padded) -> DMA-xbar transpose -> DRAM row ->
        # partition-broadcast DMA read back.  All off the critical path.
        nc.vector.tensor_copy(r_pad[:, 0:NQ], r_inv[:])
        nc.sync.dma_start_transpose(tpr[:], r_pad[:])
        nc.sync.dma_start(rr_d[0, :], tpr[0:NQ, :])
        nc.sync.dma_start(rrep[:], rr_d.broadcast_to([128, N]))
        emit_band(10)
        emit_agg(4)
        emit_band(11)
        emit_band(12)
        emit_agg(5)
        emit_band(13)
        emit_band(14)
        emit_agg(6)
        emit_band(15)
        emit_agg(7)

        # epilogue: relu (ScalarE) as each outT chunk's accumulation ends,
        # then *rrep on DVE (bf16 2x), DMA out per 512 chunk
        for c in range(4):
            sl = slice(c * 512, (c + 1) * 512)
            nc.scalar.activation(out_r[:, sl], outTs[c][:], AF.Relu)
            nc.vector.tensor_mul(out_sb[:, sl], out_r[:, sl], rrep[:, sl])
            nc.sync.dma_start(out[:, sl], out_sb[:, sl])

    nc.compile()
    return nc


_NC_CACHE = None


def _get_nc():
    global _NC_CACHE
    if _NC_CACHE is None:
        _NC_CACHE = build_nc()
    return _NC_CACHE


def make_in_maps(x, edge_weight, W, b):
    x = np.asarray(x, dtype=np.float32)
    edge_weight = np.asarray(edge_weight, dtype=np.float32)
    W = np.asarray(W, dtype=np.float32)
    b = np.asarray(b, dtype=np.float32)
    wt = np.ascontiguousarray(W.T).astype(ml_dtypes.bfloat16)
    bias = np.ascontiguousarray(b.reshape(OUT, 1))
    in_maps = []
    for core in CORE_IDS:
        in_maps.append(
            {
                "et": np.ascontiguousarray(edge_weight[core].T).astype(
                    ml_dtypes.bfloat16
                ),
                "xt": np.ascontiguousarray(x[core].T).astype(ml_dtypes.bfloat16),
                "wt": wt,
                "bias": bias,
            }
        )
    return in_maps


def kernel(x, edge_weight, W, b):
    nc = _get_nc()
    in_maps = make_in_maps(x, edge_weight, W, b)
    res = run_bass_kernel_spmd(nc, in_maps, core_ids=CORE_IDS)
    out = np.stack(
        [
            np.ascontiguousarray(res.results[i]["out"].astype(np.float32).T)
            for i in range(len(CORE_IDS))
        ]
    )
    return out


# revision 14
# speedup vs baseline: 1.0517x; 1.0517x over previous
"""Trainium2 Bass kernel for AdaptConv-style GNN message passing.

Reference computation (per batch element b):
    h   = x @ W.T + b                       # [N, OUT]
    hn  = h / max(||h||_row, 1e-12)         # row-wise L2 normalize
    cos = hn @ hn.T                         # [N, N]
    out = relu((edge_weight * cos) @ h)     # [N, OUT]

Algebraic restructure used on-chip (r_p = 1/max(||h_p||, eps)):
    out[p,:] = r_p * sum_q E[p,q] * S[p,q] * hn[q,:]
with S = h h^T the UNNORMALIZED gram.  The gram operands need no
pre-normalization (no hnT materialization, half the PE transposes); the
row scale r_p is applied in a cheap bf16 epilogue against a
row-replicated copy of r built via DMA-xbar transpose + DRAM broadcast.

Sharding: pure data-parallel over batch B=8 across 8 NeuronCores.  Host
layout preprocessing: et = edge_weight[b].T (bf16), xt = x[b].T (bf16),
wt = W.T (bf16), bias = b[:,None] (fp32); output returns as
outT = [OUT, N] bf16 and the host transposes/casts to fp32.

Per-core dataflow (fp32 PSUM accumulation everywhere):
    hT[o,n]    = wt.T @ xt + bias          (PE, 2x1024 chunks + ScalarE bias)
    h tiles    = PE-transpose(hT) -> GpSimd copy to SBUF bf16 (16x)
    norms      = ScalarE Square-accum -> Sqrt -> max/recip (chunked)
    hn8 pairs  = h * r_inv -> fp8 [128, 2*OUT] interleaved (agg weights)
    per band q (16): 4 raw-gram matmuls [128,512] (1 hT-block LDW each 4)
        gates gt8 = et * S -> fp8, split: DVE-direct / GpSimd-direct /
        2x(ScalarE psum->bf16 copy -> DVE bf16 mul)
    per band pair: 4 DoubleRow fp8 matmuls accumulate outT [OUT,2048]
    epilogue: ScalarE relu -> DVE *rrep (bf16) -> DMA out per 512 chunk

The PE is fed dummy warm-up transposes first so its DVFS p-state ramps
to 2.4 GHz before the gram matmuls start, and is kept busy end-to-end.
"""

import ml_dtypes
import numpy as np

import concourse.bass as bass
import concourse.mybir as mybir
import concourse.tile as tile
from concourse import bacc
from concourse.bass_utils import run_bass_kernel_spmd
from concourse.masks import make_identity

B, N, IN, OUT = 8, 2048, 128, 128
NQ = N // 128          # 16 row bands
NP = NQ // 2           # 8 band pairs (DoubleRow)
FP32 = mybir.dt.float32
BF16 = mybir.dt.bfloat16
FP8 = mybir.dt.float8e4
AF = mybir.ActivationFunctionType
EPS = 1e-12
WARMUP = 16            # dummy PE transposes to ramp the p-state
USE_FP8 = True         # fp8 gt/hn + DoubleRow agg (2x PE on the agg matmul)

CORE_IDS = list(range(8))


def build_nc():
    from contextlib import ExitStack

    nc = bacc.Bacc("TRN2", target_bir_lowering=False, debug=False, num_devices=8)

    et = nc.dram_tensor("et", [N, N], BF16, kind="ExternalInput").ap()
    xt = nc.dram_tensor("xt", [IN, N], BF16, kind="ExternalInput").ap()
    wt = nc.dram_tensor("wt", [IN, OUT], BF16, kind="ExternalInput").ap()
    bias = nc.dram_tensor("bias", [OUT, 1], FP32, kind="ExternalInput").ap()
    out = nc.dram_tensor("out", [OUT, N], BF16, kind="ExternalOutput").ap()
    rr_d = nc.dram_tensor("rr_d", [1, N], BF16, kind="Internal").ap()

    with tile.TileContext(nc) as tc, ExitStack() as ctx:
        singles = ctx.enter_context(tc.tile_pool(name="singles", bufs=1))
        # et stream pool directly after singles so its range never aliases
        # transient scratch (no WAR deps on the stream).
        etp = ctx.enter_context(tc.tile_pool(name="etp", bufs=16))
        gt8p = ctx.enter_context(tc.tile_pool(name="gt8p", bufs=3))
        csbp = ctx.enter_context(tc.tile_pool(name="csbp", bufs=6))
        sqp = ctx.enter_context(tc.tile_pool(name="sqp", bufs=2))

        ident = singles.tile([128, 128], BF16, tag="ident")
        make_identity(nc, ident[:])

        xt_sb = singles.tile([IN, N], BF16, tag="xt_sb")
        wt_sb = singles.tile([IN, OUT], BF16, tag="wt_sb")
        bias_sb = singles.tile([OUT, 1], FP32, tag="bias_sb")
        hT = singles.tile([128, N], BF16, tag="hT")
        hrm_all = singles.tile([128, N], BF16, tag="hrm_all")
        hrm = [hrm_all[:, i * 128 : (i + 1) * 128] for i in range(NQ)]
        sq_all = singles.tile([128, N], BF16, tag="sq_all")
        gdt = FP8 if USE_FP8 else BF16
        hn8 = [
            singles.tile([128, 2 * OUT], gdt, tag=f"hn8_{i}", name=f"hn8_{i}")
            for i in range(NP)
        ]
        s_acc = singles.tile([128, NQ], FP32, tag="s_acc")
        s_nrm = singles.tile([128, NQ], FP32, tag="s_nrm")
        s_max = singles.tile([128, NQ], FP32, tag="s_max")
        r_inv = singles.tile([128, NQ], FP32, tag="r_inv")
        r_pad = singles.tile([128, 128], BF16, tag="r_pad")
        tpr = singles.tile([128, 128], BF16, tag="tpr")
        rrep = singles.tile([128, N], BF16, tag="rrep")
        out_r = singles.tile([OUT, N], BF16, tag="out_r")
        out_sb = singles.tile([OUT, N], BF16, tag="out_sb")

        # scalar Sqrt table warm + r_pad zero, off the critical path
        dumm = sqp.tile([1, 2], FP32, tag="dumm")
        nc.gpsimd.memset(dumm[:], 1.0)
        dumm2 = sqp.tile([1, 2], FP32, tag="dumm2")
        nc.scalar.activation(dumm2[:], dumm[:], AF.Sqrt)
        nc.gpsimd.memset(r_pad[:], 0.0)

        # params head the sync DMA queue, then the et band stream
        for c in range(4):
            nc.sync.dma_start(
                xt_sb[:, c * 512 : (c + 1) * 512], xt[:, c * 512 : (c + 1) * 512]
            )
        nc.sync.dma_start(wt_sb[:], wt)
        nc.sync.dma_start(bias_sb[:], bias)
        etbs = []
        for q in range(NQ):
            etb = etp.tile([128, N], BF16, tag="etb", name=f"etb{q}")
            nc.sync.dma_start(etb[:], et[q * 128 : (q + 1) * 128, :])
            etbs.append(etb)

        # ---------- prologue psum (scoped; closes before main pools) ----------
        with ExitStack() as pctx:
            warm = pctx.enter_context(tc.tile_pool(name="warm", bufs=1, space="PSUM"))
            hps_pool = pctx.enter_context(
                tc.tile_pool(name="hps", bufs=3, space="PSUM")
            )
            tpp = pctx.enter_context(tc.tile_pool(name="tpp", bufs=4, space="PSUM"))

            # PE p-state warm-up while waiting for xt: harmless transposes
            wtile = warm.tile([128, 128], BF16, tag="wtile")
            for _ in range(WARMUP):
                nc.tensor.transpose(wtile[:], ident[:], ident[:])

            # hT = wt.T @ xt + bias (four 512-wide chunks; first gram matmul
            # can start as soon as bias chunk 0 lands)
            hps = []
            for c in range(3):
                ps = hps_pool.tile([OUT, 512], FP32, tag="hps", name=f"hps{c}")
                nc.tensor.matmul(
                    ps[:], wt_sb[:], xt_sb[:, c * 512 : (c + 1) * 512],
                    start=True, stop=True,
                )
                hps.append(ps)
            # keep the PE p-state alive while the first bias chunk drains
            for _ in range(3):
                nc.tensor.transpose(wtile[:], ident[:], ident[:])
            ps3 = hps_pool.tile([OUT, 512], FP32, tag="hps", name="hps3")
            nc.tensor.matmul(
                ps3[:], wt_sb[:], xt_sb[:, 3 * 512 : 4 * 512], start=True, stop=True
            )
            hps.append(ps3)
            for c in range(4):
                nc.scalar.activation(
                    hT[:, c * 512 : (c + 1) * 512], hps[c][:], AF.Identity,
                    bias=bias_sb[:], scale=1.0,
                )

            # row-major h tiles: PE transpose -> DVE/ScalarE copy to SBUF bf16
            for i in range(NQ):
                tp = tpp.tile([128, 128], BF16, tag="tp", name=f"tp{i}")
                nc.tensor.transpose(tp[:], hT[:, i * 128 : (i + 1) * 128], ident[:])
                if i < 8:
                    nc.vector.tensor_copy(hrm[i], tp[:])
                else:
                    nc.scalar.copy(hrm[i], tp[:])

        # ---------- main psum pools ----------
        cosp = ctx.enter_context(tc.tile_pool(name="cosp", bufs=4, space="PSUM"))
        outp = ctx.enter_context(tc.tile_pool(name="outp", bufs=1, space="PSUM"))
        outTs = [
            outp.tile([OUT, 512], FP32, tag=f"outT{c}", name=f"outT{c}")
            for c in range(4)
        ]

        def emit_sqb(lo, hi):
            # batched row-norm^2 for bands [lo,hi): one bf16 mul (2x mode)
            # + one 3D-AP reduce over [128, k, 128] -> [128, k]
            sl = slice(lo * 128, hi * 128)
            nc.vector.tensor_mul(sq_all[:, sl], hrm_all[:, sl], hrm_all[:, sl])
            nc.vector.tensor_reduce(
                s_acc[:, lo:hi],
                sq_all[:, sl].rearrange("p (i c) -> p i c", c=128),
                mybir.AxisListType.X, mybir.AluOpType.add,
            )

        def emit_rfin(ck):
            nc.scalar.activation(s_nrm[:, ck], s_acc[:, ck], AF.Sqrt)
            nc.vector.tensor_scalar_max(s_max[:, ck], s_nrm[:, ck], EPS)
            nc.vector.reciprocal(r_inv[:, ck], s_max[:, ck])

        def emit_hn8(i, eng="s"):
            dst = hn8[i // 2][:, (i % 2) * OUT : (i % 2 + 1) * OUT]
            if eng == "s":
                nc.scalar.mul(dst, hrm[i], r_inv[:, i : i + 1])
            else:
                nc.vector.tensor_scalar_mul(dst, hrm[i], r_inv[:, i : i + 1])

        gt8s = [None] * NP

        def emit_band(q):
            """raw-gram matmuls + gates for band q."""
            if q % 2 == 0:
                gt8s[q // 2] = gt8p.tile([128, 2 * N], gdt, tag="gt8", name=f"gt8_{q//2}")
            gt = gt8s[q // 2]
            ko = (q % 2) * N
            cps = []
            for c in range(4):
                cp = cosp.tile([128, 512], FP32, tag="cps", name=f"cps{q}_{c}")
                nc.tensor.matmul(
                    cp[:], hT[:, q * 128 : (q + 1) * 128],
                    hT[:, c * 512 : (c + 1) * 512],
                    start=True, stop=True,
                )
                cps.append(cp)
            # gate split: DVE-direct is the cheapest single-op path (1x from
            # fp32 psum); the rest go ScalarE-copy -> GpSimd bf16 mul.  fp8
            # output drops DVE TT to 1x, so a copy+DVE-mul path buys nothing.
            def gate_v(c):
                nc.vector.tensor_mul(
                    gt[:, ko + c * 512 : ko + (c + 1) * 512],
                    cps[c][:], etbs[q][:, c * 512 : (c + 1) * 512],
                )

            def gate_sg(c):
                csb = csbp.tile([128, 512], BF16, tag="csb", name=f"csb{c}_{q}")
                nc.scalar.copy(csb[:], cps[c][:])
                nc.gpsimd.tensor_mul(
                    gt[:, ko + c * 512 : ko + (c + 1) * 512],
                    csb[:], etbs[q][:, c * 512 : (c + 1) * 512],
                )

            gate_v(0)
            gate_sg(1)
            if q % 2 == 0:
                gate_v(2)
            else:
                gate_sg(2)
            if q % 4 == 3:
                gate_sg(3)
            else:
                gate_v(3)

        def emit_agg(p):
            if USE_FP8:
                lhs = hn8[p][:].rearrange("q (k m) -> q k m", k=2)
                rhs = gt8s[p][:].rearrange("q (k n) -> q k n", k=2)
                for c in range(4):
                    nc.tensor.matmul(
                        outTs[c][:], lhs, rhs[:, :, c * 512 : (c + 1) * 512],
                        start=(p == 0), stop=(p == NP - 1),
                        perf_mode=mybir.MatmulPerfMode.DoubleRow,
                    )
            else:
                for ko in range(2):
                    for c in range(4):
                        nc.tensor.matmul(
                            outTs[c][:],
                            hn8[p][:, ko * OUT : (ko + 1) * OUT],
                            gt8s[p][:, ko * N + c * 512 : ko * N + (c + 1) * 512],
                            start=(p == 0 and ko == 0),
                            stop=(p == NP - 1 and ko == 1),
                        )

        emit_band(0)
        emit_sqb(0, 2)
        emit_rfin(slice(0, 2))
        emit_hn8(0, "v")
        emit_hn8(1, "v")
        emit_band(1)
        emit_sqb(2, 8)
        emit_rfin(slice(2, 8))
        emit_band(2)
        emit_agg(0)
        emit_hn8(2)
        emit_hn8(3)
        emit_band(3)
        emit_hn8(4)
        emit_hn8(5)
        emit_sqb(8, 16)
        emit_rfin(slice(8, 16))
        emit_band(4)
        emit_agg(1)
        emit_hn8(6)
        emit_hn8(7)
        emit_band(5)
        for i in range(8, 12):
            emit_hn8(i)
        emit_band(6)
        emit_agg(2)
        for i in range(12, 16):
            emit_hn8(i)
        emit_band(7)
        emit_band(8)
        emit_agg(3)
        # rrep: r_inv -> bf16 (padded) -> DMA-xbar transpose -> DRAM row ->
        # partition-broadcast DMA read back.  All off the critical path.
        nc.vector.tensor_copy(r_pad[:, 0:NQ], r_inv[:])
        nc.sync.dma_start_transpose(tpr[:], r_pad[:])
        nc.sync.dma_start(rr_d[0, :], tpr[0:NQ, :])
        nc.sync.dma_start(rrep[:], rr_d.broadcast_to([128, N]))
        emit_band(9)
        emit_band(10)
        emit_agg(4)
        emit_band(11)
        emit_band(12)
        emit_agg(5)
        emit_band(13)
        emit_band(14)
        emit_agg(6)
        emit_band(15)
        emit_agg(7)

        # epilogue: relu (ScalarE) as each outT chunk's accumulation ends,
        # then *rrep on DVE (bf16 2x), DMA out per 512 chunk
        for c in range(4):
            sl = slice(c * 512, (c + 1) * 512)
            nc.scalar.activation(out_r[:, sl], outTs[c][:], AF.Relu)
            nc.vector.tensor_mul(out_sb[:, sl], out_r[:, sl], rrep[:, sl])
            nc.sync.dma_start(out[:, sl], out_sb[:, sl])

    nc.compile()
    return nc


_NC_CACHE = None


def _get_nc():
    global _NC_CACHE
    if _NC_CACHE is None:
        _NC_CACHE = build_nc()
    return _NC_CACHE


def make_in_maps(x, edge_weight, W, b):
    x = np.asarray(x, dtype=np.float32)
    edge_weight = np.asarray(edge_weight, dtype=np.float32)
    W = np.asarray(W, dtype=np.float32)
    b = np.asarray(b, dtype=np.float32)
    wt = np.ascontiguousarray(W.T).astype(ml_dtypes.bfloat16)
    bias = np.ascontiguousarray(b.reshape(OUT, 1))
    in_maps = []
    for core in CORE_IDS:
        in_maps.append(
            {
                "et": np.ascontiguousarray(edge_weight[core].T).astype(
                    ml_dtypes.bfloat16
                ),
                "xt": np.ascontiguousarray(x[core].T).astype(ml_dtypes.bfloat16),
                "wt": wt,
                "bias": bias,
            }
        )
    return in_maps


def kernel(x, edge_weight, W, b):
    nc = _get_nc()
    in_maps = make_in_maps(x, edge_weight, W, b)
    res = run_bass_kernel_spmd(nc, in_maps, core_ids=CORE_IDS)
    out = np.stack(
        [
            np.ascontiguousarray(res.results[i]["out"].astype(np.float32).T)
            for i in range(len(CORE_IDS))
        ]
    )
    return out
